# revision 10
# baseline (speedup 1.0000x reference)
"""DotAttentionLayer (head_dim=1) Trainium2 kernel.

Math (per batch b):
    scores[i, j] = q[i] * k[j] / 64          (outer product; sqrt(4096) = 64)
    dist = softmax_j(scores)                 -> [N, N] written to HBM (the bulk)
    out[i] = sum_j dist[i, j] * v[j]

Key structure exploited: scores are rank-1, so the softmax normalizer
    Z(t) = sum_j exp(t * k_j),  t = q_i / 64
is an analytic function of the scalar t. With |t * k_j| <= ~0.36, the Taylor
series Z(t) = sum_m S_m t^m / m! (S_m = sum_j k_j^m) converges to machine
precision by m ~ 10, and likewise the output numerator W(t) = sum_j exp(t k_j) v_j
with moments T_m = sum_j v_j k_j^m. So the O(N^2) softmax collapses to:

    precompute (O(N), tiny): moments S_m -> Z_i -> -lnZ_i  (DVE + one PE matmul;
                             -lnZ via ln(1+u) series since Z/4096 = 1+u, |u|<3e-3)
    main loop (O(N^2)):      dist[i, j] = exp(q_i/64 * k_j - lnZ_i)
                             = ONE ScalarE activation per 128-row tile with
                               per-partition scale (q_i/64) and bias (-lnZ_i),
                             + ONE full-128-partition 2MB DMA store per tile
    output (O(N), tiny):     T_m moments -> W_i -> out_i = W_i / Z_i (deferred
                             off the critical path)

The main loop is ScalarE exp (~3.8us / [128,4096] tile) fully overlapped with
the HBM write of the 1 GiB distribution. Dist stores use one dma_start per
[128, 4096] tile: a single full-partition 2MB descriptor chain fans out across
all 16 SDMA engines and sustains ~407 GB/s (splitting by partitions serializes
on the shared ring set and drops to ~240 GB/s).

Sharding: B=16 batches, pure data parallel, 2 batches per core across 8 cores.
"""

import math
import sys

import numpy as np

for _p in ("/opt/trn_rl_repo",):
    if _p not in sys.path:
        sys.path.insert(0, _p)

import concourse.bacc as bacc
import concourse.bass as bass
import concourse.mybir as mybir
import concourse.tile as tile
from concourse.bass_utils import run_bass_kernel_spmd

B, N = 16, 4096
N_CORES = 8
BPC = B // N_CORES  # batches per core = 2
P = 128             # partitions
F = N // P          # 32 columns per row-tile in the natural [128, 32] layout
M = 10              # Taylor terms m = 0..9 (remainder ~1e-11 relative)
INV_SQRT_SCALE = 1.0 / 64.0
LN_N = float(np.log(np.float32(N)))
F32 = mybir.dt.float32

DIST_BUFS = 6       # dist tile buffering depth

_TRACE = False      # test.py flips this to capture an NTFF/perfetto profile
_LAST_RESULTS = None


def _emit(tc):
    nc = tc.nc
    q_in = nc.dram_tensor("query", [BPC, N], F32, kind="ExternalInput").ap()
    v_in = nc.dram_tensor("value", [BPC, N], F32, kind="ExternalInput").ap()
    k_in = nc.dram_tensor("key", [BPC, N], F32, kind="ExternalInput").ap()
    dist_out = nc.dram_tensor("dist", [BPC, N, N], F32, kind="ExternalOutput").ap()
    out_out = nc.dram_tensor("out", [BPC, N], F32, kind="ExternalOutput").ap()

    import contextlib

    with contextlib.ExitStack() as ctx:
        small = ctx.enter_context(tc.tile_pool(name="small", bufs=2))
        consts = ctx.enter_context(tc.tile_pool(name="consts", bufs=1))
        kb_pool = ctx.enter_context(tc.tile_pool(name="kb", bufs=2))
        psum = ctx.enter_context(tc.tile_pool(name="psum", bufs=2, space="PSUM"))
        dist_pool = ctx.enter_context(tc.tile_pool(name="dist", bufs=DIST_BUFS))

        # shared constants
        ones_col = consts.tile([P, 1], F32)
        nc.vector.memset(ones_col, 1.0)
        ones_row = consts.tile([1, P], F32)
        nc.vector.memset(ones_row, 1.0)
        fact_row = consts.tile([1, M], F32)
        for m in range(M):
            nc.vector.memset(fact_row[:, m : m + 1], 1.0 / math.factorial(m))

        zpath = []  # per-batch tiles needed later
        # ---------- Z-critical path: everything gating the first ACT exp ----------
        for b in range(BPC):
            k_nat = small.tile([P, F], F32, tag="k_nat")
            q_nat = small.tile([P, F], F32, tag="q_nat")
            nc.sync.dma_start(out=k_nat, in_=k_in[b].rearrange("(p f) -> p f", p=P))
            nc.sync.dma_start(out=q_nat, in_=q_in[b].rearrange("(p f) -> p f", p=P))
            k_bcast = kb_pool.tile([P, N], F32, tag="k_bcast")
            nc.sync.dma_start(out=k_bcast, in_=k_in[b].partition_broadcast(P))

            qs = small.tile([P, F], F32, tag="qs")
            nc.vector.tensor_scalar_mul(qs, q_nat, INV_SQRT_SCALE)

            # POW[:, m*F:(m+1)*F] = k^m (raw powers; 1/m! folded into moments)
            POW = small.tile([P, M * F], F32, tag="POW")

            def pw(m):
                return POW[:, m * F : (m + 1) * F]

            nc.vector.memset(pw(0), 1.0)
            nc.vector.tensor_copy(pw(1), k_nat)
            nc.vector.tensor_mul(pw(2), k_nat, k_nat)
            # even/odd chains off k^2 to halve serial depth
            for m in range(3, M):
                nc.vector.tensor_mul(pw(m), pw(m - 2), pw(2))

            # S_m = column sums over all partitions (PE ones-matmul), scaled 1/m!
            psum_s = psum.tile([1, M * F], F32, tag="psum_s")
            nc.tensor.matmul(psum_s, ones_col, POW, start=True, stop=True)
            srow = small.tile([1, M], F32, tag="srow")
            nc.vector.tensor_reduce(
                srow,
                psum_s.rearrange("p (m f) -> p m f", f=F),
                axis=mybir.AxisListType.X,
                op=mybir.AluOpType.add,
            )
            nc.vector.tensor_mul(srow, srow, fact_row)
            # broadcast moments to all partitions
            psum_bs = psum.tile([P, M], F32, tag="psum_bs")
            nc.tensor.matmul(psum_bs, ones_row, srow, start=True, stop=True)
            MOMS = small.tile([P, M], F32, tag="MOMS")
            nc.vector.tensor_copy(MOMS, psum_bs)

            # Horner: Z = sum_m S_m/m! * qs^m
            Z = small.tile([P, F], F32, tag="Z")
            tmpZ = small.tile([P, F], F32, tag="tmpZ")
            nc.vector.tensor_scalar_mul(Z, pw(0), MOMS[:, M - 1 : M])
            for m in range(M - 2, -1, -1):
                nc.vector.tensor_mul(tmpZ, Z, qs)
                nc.vector.tensor_scalar_add(Z, tmpZ, MOMS[:, m : m + 1])

            # -lnZ = -ln(N) - ln(1+u), u = Z/N - 1 (|u| < 3e-3)
            u = small.tile([P, F], F32, tag="u")
            nc.vector.tensor_scalar(
                u, Z, 1.0 / N, -1.0,
                op0=mybir.AluOpType.mult, op1=mybir.AluOpType.add,
            )
            pn = small.tile([P, F], F32, tag="pn")
            tmpn = small.tile([P, F], F32, tag="tmpn")
            # p = ((-u/4 + 1/3)*u - 1/2)*u + 1 ; ln1p = p*u
            nc.vector.tensor_scalar(
                pn, u, -0.25, 1.0 / 3.0,
                op0=mybir.AluOpType.mult, op1=mybir.AluOpType.add,
            )
            nc.vector.tensor_mul(tmpn, pn, u)
            nc.vector.tensor_scalar_add(pn, tmpn, -0.5)
            nc.vector.tensor_mul(tmpn, pn, u)
            nc.vector.tensor_scalar_add(pn, tmpn, 1.0)
            nc.vector.tensor_mul(tmpn, pn, u)
            neglnZ = small.tile([P, F], F32, tag="neglnZ")
            nc.vector.tensor_scalar(
                neglnZ, tmpn, -1.0, -LN_N,
                op0=mybir.AluOpType.mult, op1=mybir.AluOpType.add,
            )
            zpath.append((k_bcast, qs, neglnZ, POW, Z))

        # ---------- main loop: 32 tiles per batch, ACT exp + one 2MB DMA each ----------
        # Alternate stores between the two HWDGE rings (SP + ACT issuing
        # engines) so one ring's trigger/completion turnaround overlaps the
        # other ring's transfer.
        for b in reversed(range(BPC)):
            k_bcast, qs, neglnZ, _, _ = zpath[b]
            # tile t holds rows i = 32p + t on partition p
            dview = dist_out[b].rearrange("(p f) n -> p f n", p=P)
            for t in range(F):
                dt = dist_pool.tile([P, N], F32, tag="dt")
                nc.scalar.activation(
                    dt,
                    k_bcast,
                    mybir.ActivationFunctionType.Exp,
                    bias=neglnZ[:, t : t + 1],
                    scale=qs[:, t : t + 1],
                )
                eng = nc.sync if t % 2 == 0 else nc.scalar
                eng.dma_start(out=dview[:, t, :], in_=dt)

        # ---------- deferred output path (off the DMA-critical path) ----------
        for b in range(BPC):
            _, qs, _, POW, Z = zpath[b]
            v_nat = small.tile([P, F], F32, tag="v_nat")
            nc.sync.dma_start(out=v_nat, in_=v_in[b].rearrange("(p f) -> p f", p=P))
            VP = small.tile([P, M * F], F32, tag="VP")
            for m in range(M):
                nc.vector.tensor_mul(
                    VP[:, m * F : (m + 1) * F], POW[:, m * F : (m + 1) * F], v_nat
                )
            psum_t = psum.tile([1, M * F], F32, tag="psum_t")
            nc.tensor.matmul(psum_t, ones_col, VP, start=True, stop=True)
            trow = small.tile([1, M], F32, tag="trow")
            nc.vector.tensor_reduce(
                trow,
                psum_t.rearrange("p (m f) -> p m f", f=F),
                axis=mybir.AxisListType.X,
                op=mybir.AluOpType.add,
            )
            nc.vector.tensor_mul(trow, trow, fact_row)
            psum_bt = psum.tile([P, M], F32, tag="psum_bt")
            nc.tensor.matmul(psum_bt, ones_row, trow, start=True, stop=True)
            MOMT = small.tile([P, M], F32, tag="MOMT")
            nc.vector.tensor_copy(MOMT, psum_bt)

            W = small.tile([P, F], F32, tag="W")
            tmpW = small.tile([P, F], F32, tag="tmpW")
            nc.vector.tensor_scalar_mul(W, POW[:, 0:F], MOMT[:, M - 1 : M])
            for m in range(M - 2, -1, -1):
                nc.vector.tensor_mul(tmpW, W, qs)
                nc.vector.tensor_scalar_add(W, tmpW, MOMT[:, m : m + 1])
            rZ = small.tile([P, F], F32, tag="rZ")
            nc.vector.reciprocal(rZ, Z)
            out_t = small.tile([P, F], F32, tag="out_t")
            nc.vector.tensor_mul(out_t, W, rZ)
            nc.sync.dma_start(
                out=out_out[b].rearrange("(p f) -> p f", p=P), in_=out_t
            )


_nc_cache = None


def _get_nc():
    global _nc_cache
    if _nc_cache is None:
        nc = bacc.Bacc("TRN2", target_bir_lowering=False, debug=False)
        with tile.TileContext(nc) as tc:
            _emit(tc)
        nc.compile()
        _nc_cache = nc
    return _nc_cache


def kernel(query, value, key):
    global _LAST_RESULTS
    q = np.ascontiguousarray(np.asarray(query, dtype=np.float32))
    v = np.ascontiguousarray(np.asarray(value, dtype=np.float32))
    k = np.ascontiguousarray(np.asarray(key, dtype=np.float32))
    assert q.shape == (B, N) and v.shape == (B, N) and k.shape == (B, N)

    nc = _get_nc()
    in_maps = [
        {
            "query": q[c * BPC : (c + 1) * BPC],
            "value": v[c * BPC : (c + 1) * BPC],
            "key": k[c * BPC : (c + 1) * BPC],
        }
        for c in range(N_CORES)
    ]
    res = run_bass_kernel_spmd(
        nc, in_maps, core_ids=list(range(N_CORES)), trace=_TRACE
    )
    _LAST_RESULTS = res
    out = np.concatenate([res.results[c]["out"] for c in range(N_CORES)], axis=0)
    dist = np.concatenate([res.results[c]["dist"] for c in range(N_CORES)], axis=0)
    return out, dist


# revision 11
# speedup vs baseline: 1.1735x; 1.1735x over previous
"""DotAttentionLayer (head_dim=1) Trainium2 kernel.

Math (per batch b):
    scores[i, j] = q[i] * k[j] / 64          (outer product; sqrt(4096) = 64)
    dist = softmax_j(scores)                 -> [N, N] written to HBM (the bulk)
    out[i] = sum_j dist[i, j] * v[j]

Key structure exploited: scores are rank-1, so the softmax normalizer
    Z(t) = sum_j exp(t * k_j),  t = q_i / 64
is an analytic function of the scalar t. With |t * k_j| <= ~0.36, the Taylor
series Z(t) = sum_m S_m t^m / m! (S_m = sum_j k_j^m) converges to machine
precision by m ~ 10, and likewise the output numerator W(t) = sum_j exp(t k_j) v_j
with moments T_m = sum_j v_j k_j^m. So the O(N^2) softmax collapses to:

    precompute (O(N), tiny): moments S_m -> Z_i -> -lnZ_i  (DVE + one PE matmul;
                             -lnZ via ln(1+u) series since Z/4096 = 1+u, |u|<3e-3)
    main loop (O(N^2)):      dist[i, j] = exp(q_i/64 * k_j - lnZ_i)
                             = ONE ScalarE activation per 128-row tile with
                               per-partition scale (q_i/64) and bias (-lnZ_i),
                             + ONE full-128-partition 2MB DMA store per tile
    output (O(N), tiny):     T_m moments -> W_i -> out_i = W_i / Z_i (deferred
                             off the critical path)

The main loop is ScalarE exp (~3.8us / [128,4096] tile) fully overlapped with
the HBM write of the 1 GiB distribution. Dist stores use one dma_start per
[128, 4096] tile: a single full-partition 2MB descriptor chain fans out across
all 16 SDMA engines and sustains ~407 GB/s (splitting by partitions serializes
on the shared ring set and drops to ~240 GB/s).

Sharding: B=16 batches, pure data parallel, 2 batches per core across 8 cores.
"""

import math
import sys

import numpy as np

for _p in ("/opt/trn_rl_repo",):
    if _p not in sys.path:
        sys.path.insert(0, _p)

import concourse.bacc as bacc
import concourse.bass as bass
import concourse.mybir as mybir
import concourse.tile as tile
from concourse.bass_utils import run_bass_kernel_spmd

B, N = 16, 4096
N_CORES = 8
BPC = B // N_CORES  # batches per core = 2
P = 128             # partitions
F = N // P          # 32 columns per row-tile in the natural [128, 32] layout
M = 10              # Taylor terms m = 0..9 (remainder ~1e-11 relative)
INV_SQRT_SCALE = 1.0 / 64.0
LN_N = float(np.log(np.float32(N)))
F32 = mybir.dt.float32

DIST_BUFS = 6       # dist tile buffering depth

_TRACE = False      # test.py flips this to capture an NTFF/perfetto profile
_LAST_RESULTS = None


def _emit(tc):
    nc = tc.nc
    q_in = nc.dram_tensor("query", [BPC, N], F32, kind="ExternalInput").ap()
    v_in = nc.dram_tensor("value", [BPC, N], F32, kind="ExternalInput").ap()
    k_in = nc.dram_tensor("key", [BPC, N], F32, kind="ExternalInput").ap()
    dist_out = nc.dram_tensor("dist", [BPC, N, N], F32, kind="ExternalOutput").ap()
    out_out = nc.dram_tensor("out", [BPC, N], F32, kind="ExternalOutput").ap()

    import contextlib

    with contextlib.ExitStack() as ctx:
        small = ctx.enter_context(tc.tile_pool(name="small", bufs=2))
        consts = ctx.enter_context(tc.tile_pool(name="consts", bufs=1))
        kb_pool = ctx.enter_context(tc.tile_pool(name="kb", bufs=2))
        psum = ctx.enter_context(tc.tile_pool(name="psum", bufs=2, space="PSUM"))
        dist_pool = ctx.enter_context(tc.tile_pool(name="dist", bufs=DIST_BUFS))

        # shared constants
        ones_col = consts.tile([P, 1], F32)
        nc.vector.memset(ones_col, 1.0)
        ones_row = consts.tile([1, P], F32)
        nc.vector.memset(ones_row, 1.0)
        fact_row = consts.tile([1, M], F32)
        for m in range(M):
            nc.vector.memset(fact_row[:, m : m + 1], 1.0 / math.factorial(m))

        zpath = []  # per-batch tiles needed later
        # ---------- Z-critical path: everything gating the first ACT exp ----------
        for b in range(BPC):
            k_nat = small.tile([P, F], F32, tag="k_nat")
            q_nat = small.tile([P, F], F32, tag="q_nat")
            nc.sync.dma_start(out=k_nat, in_=k_in[b].rearrange("(p f) -> p f", p=P))
            nc.sync.dma_start(out=q_nat, in_=q_in[b].rearrange("(p f) -> p f", p=P))
            k_bcast = kb_pool.tile([P, N], F32, tag="k_bcast")
            nc.sync.dma_start(out=k_bcast, in_=k_in[b].partition_broadcast(P))

            qs = small.tile([P, F], F32, tag="qs")
            nc.vector.tensor_scalar_mul(qs, q_nat, INV_SQRT_SCALE)

            # POW[:, m*F:(m+1)*F] = k^m (raw powers; 1/m! folded into moments)
            POW = small.tile([P, M * F], F32, tag="POW")

            def pw(m):
                return POW[:, m * F : (m + 1) * F]

            nc.vector.memset(pw(0), 1.0)
            nc.vector.tensor_copy(pw(1), k_nat)
            nc.vector.tensor_mul(pw(2), k_nat, k_nat)
            # even/odd chains off k^2 to halve serial depth
            for m in range(3, M):
                nc.vector.tensor_mul(pw(m), pw(m - 2), pw(2))

            # S_m = column sums over all partitions (PE ones-matmul), scaled 1/m!
            psum_s = psum.tile([1, M * F], F32, tag="psum_s")
            nc.tensor.matmul(psum_s, ones_col, POW, start=True, stop=True)
            srow = small.tile([1, M], F32, tag="srow")
            nc.vector.tensor_reduce(
                srow,
                psum_s.rearrange("p (m f) -> p m f", f=F),
                axis=mybir.AxisListType.X,
                op=mybir.AluOpType.add,
            )
            nc.vector.tensor_mul(srow, srow, fact_row)
            # broadcast moments to all partitions
            psum_bs = psum.tile([P, M], F32, tag="psum_bs")
            nc.tensor.matmul(psum_bs, ones_row, srow, start=True, stop=True)
            MOMS = small.tile([P, M], F32, tag="MOMS")
            nc.vector.tensor_copy(MOMS, psum_bs)

            # Horner: Z = sum_m S_m/m! * qs^m
            Z = small.tile([P, F], F32, tag="Z")
            tmpZ = small.tile([P, F], F32, tag="tmpZ")
            nc.vector.tensor_scalar_mul(Z, pw(0), MOMS[:, M - 1 : M])
            for m in range(M - 2, -1, -1):
                nc.vector.tensor_mul(tmpZ, Z, qs)
                nc.vector.tensor_scalar_add(Z, tmpZ, MOMS[:, m : m + 1])

            # -lnZ = -ln(N) - ln(1+u), u = Z/N - 1 (|u| < 3e-3)
            u = small.tile([P, F], F32, tag="u")
            nc.vector.tensor_scalar(
                u, Z, 1.0 / N, -1.0,
                op0=mybir.AluOpType.mult, op1=mybir.AluOpType.add,
            )
            pn = small.tile([P, F], F32, tag="pn")
            tmpn = small.tile([P, F], F32, tag="tmpn")
            # p = ((-u/4 + 1/3)*u - 1/2)*u + 1 ; ln1p = p*u
            nc.vector.tensor_scalar(
                pn, u, -0.25, 1.0 / 3.0,
                op0=mybir.AluOpType.mult, op1=mybir.AluOpType.add,
            )
            nc.vector.tensor_mul(tmpn, pn, u)
            nc.vector.tensor_scalar_add(pn, tmpn, -0.5)
            nc.vector.tensor_mul(tmpn, pn, u)
            nc.vector.tensor_scalar_add(pn, tmpn, 1.0)
            nc.vector.tensor_mul(tmpn, pn, u)
            neglnZ = small.tile([P, F], F32, tag="neglnZ")
            nc.vector.tensor_scalar(
                neglnZ, tmpn, -1.0, -LN_N,
                op0=mybir.AluOpType.mult, op1=mybir.AluOpType.add,
            )
            zpath.append((k_bcast, qs, neglnZ, POW, Z))

        # ---------- main loop: 32 tiles per batch, ACT exp + one 2MB DMA each ----------
        # Alternate stores between the two HWDGE rings (SP + ACT issuing
        # engines) so one ring's trigger/completion turnaround overlaps the
        # other ring's transfer.
        dviews = [dist_out[b].rearrange("(p f) n -> p f n", p=P) for b in range(BPC)]
        for idx, (b, t) in enumerate(
            (tb % BPC, tb // BPC) for tb in range(BPC * F)
        ):
            k_bcast, qs, neglnZ, _, _ = zpath[b]
            dt = dist_pool.tile([P, N], F32, tag="dt")
            nc.scalar.activation(
                dt,
                k_bcast,
                mybir.ActivationFunctionType.Exp,
                bias=neglnZ[:, t : t + 1],
                scale=qs[:, t : t + 1],
            )
            eng = nc.sync if idx % 2 == 0 else nc.scalar
            eng.dma_start(out=dviews[b][:, t, :], in_=dt)

        # ---------- deferred output path (off the DMA-critical path) ----------
        for b in range(BPC):
            _, qs, _, POW, Z = zpath[b]
            v_nat = small.tile([P, F], F32, tag="v_nat")
            nc.sync.dma_start(out=v_nat, in_=v_in[b].rearrange("(p f) -> p f", p=P))
            VP = small.tile([P, M * F], F32, tag="VP")
            for m in range(M):
                nc.vector.tensor_mul(
                    VP[:, m * F : (m + 1) * F], POW[:, m * F : (m + 1) * F], v_nat
                )
            psum_t = psum.tile([1, M * F], F32, tag="psum_t")
            nc.tensor.matmul(psum_t, ones_col, VP, start=True, stop=True)
            trow = small.tile([1, M], F32, tag="trow")
            nc.vector.tensor_reduce(
                trow,
                psum_t.rearrange("p (m f) -> p m f", f=F),
                axis=mybir.AxisListType.X,
                op=mybir.AluOpType.add,
            )
            nc.vector.tensor_mul(trow, trow, fact_row)
            psum_bt = psum.tile([P, M], F32, tag="psum_bt")
            nc.tensor.matmul(psum_bt, ones_row, trow, start=True, stop=True)
            MOMT = small.tile([P, M], F32, tag="MOMT")
            nc.vector.tensor_copy(MOMT, psum_bt)

            W = small.tile([P, F], F32, tag="W")
            tmpW = small.tile([P, F], F32, tag="tmpW")
            nc.vector.tensor_scalar_mul(W, POW[:, 0:F], MOMT[:, M - 1 : M])
            for m in range(M - 2, -1, -1):
                nc.vector.tensor_mul(tmpW, W, qs)
                nc.vector.tensor_scalar_add(W, tmpW, MOMT[:, m : m + 1])
            rZ = small.tile([P, F], F32, tag="rZ")
            nc.vector.reciprocal(rZ, Z)
            out_t = small.tile([P, F], F32, tag="out_t")
            nc.vector.tensor_mul(out_t, W, rZ)
            nc.sync.dma_start(
                out=out_out[b].rearrange("(p f) -> p f", p=P), in_=out_t
            )


_nc_cache = None


def _get_nc():
    global _nc_cache
    if _nc_cache is None:
        nc = bacc.Bacc("TRN2", target_bir_lowering=False, debug=False)
        with tile.TileContext(nc) as tc:
            _emit(tc)
        nc.compile()
        _nc_cache = nc
    return _nc_cache


def kernel(query, value, key):
    global _LAST_RESULTS
    q = np.ascontiguousarray(np.asarray(query, dtype=np.float32))
    v = np.ascontiguousarray(np.asarray(value, dtype=np.float32))
    k = np.ascontiguousarray(np.asarray(key, dtype=np.float32))
    assert q.shape == (B, N) and v.shape == (B, N) and k.shape == (B, N)

    nc = _get_nc()
    in_maps = [
        {
            "query": q[c * BPC : (c + 1) * BPC],
            "value": v[c * BPC : (c + 1) * BPC],
            "key": k[c * BPC : (c + 1) * BPC],
        }
        for c in range(N_CORES)
    ]
    res = run_bass_kernel_spmd(
        nc, in_maps, core_ids=list(range(N_CORES)), trace=_TRACE
    )
    _LAST_RESULTS = res
    out = np.concatenate([res.results[c]["out"] for c in range(N_CORES)], axis=0)
    dist = np.concatenate([res.results[c]["dist"] for c in range(N_CORES)], axis=0)
    return out, dist
